# revision 3
# baseline (speedup 1.0000x reference)
"""DiffAttn kernel for 8 Trainium2 NeuronCores.

Sharding: core c -> (batch b = c//2, query-half h = c%2). Each core computes
2048 query rows of both score matrices against the full K/V of its batch.

Host-side prep (inside kernel()): X/W are cast to fp16 and transposed to
contraction-major layout on the host, so the device does no cast/transpose
preamble at all. Per core the two sequence halves of X.T are rotated so the
core's own query rows always form the prefix (key order is permutation
invariant under softmax + P@V, queries must be the fixed prefix for SPMD).

Per-core pipeline (all matmul inputs fp16, fp32 PSUM accumulate):
  1. Straight DMA loads of W.T / X.T tiles (no xbar, no casts).
  2. PE projections -> QT/KT [feature, seq] fp16, V [seq, d] fp16; V bias via
     DVE add of a host-broadcast row.
  3. Per 512-row query block: both exp-score matrices E1/E2 [k, q] stay
     resident in SBUF; DVE row-sum accumulation; then a SINGLE P@V matmul
     pass over D = E1 - (lam*s1/s2) * E2 (per-query factor broadcast along
     partitions via a tiny transpose-matmul + gpsimd partition_broadcast),
     with the remaining 1/s1 scale applied per-partition on the output.
     This halves the P@V matmul cost versus normalizing each matrix.
"""

import math
import os

import numpy as np

import concourse.bacc as bacc
import concourse.mybir as mybir
import concourse.tile as tile
from concourse.bass_utils import run_bass_kernel_spmd

F32 = mybir.dt.float32
F16 = mybir.dt.float16
AF = mybir.ActivationFunctionType
ALU = mybir.AluOpType

B, S, E, D = 4, 4096, 1024, 512
TWO_D = 2 * D
QR = S // 2          # query rows per core
QB = 512             # query block in attention
P = 128
N_E = E // P         # 8 contraction chunks over E
N_F = TWO_D // P     # 8 feature chunks for Q/K
KC = S // P          # 32 key chunks
NQS = QB // P        # 4 query sub-blocks per q-block
LAMBDA_INIT = 0.05
S_SCALE = 1.0 / math.sqrt(D)

LAST_RESULTS = None


def _emit(nc, tc, ctx, reps=1):
    XT = nc.dram_tensor("XT", [E, S], F16, kind="ExternalInput").ap()
    WqT = nc.dram_tensor("WqT", [E, TWO_D], F16, kind="ExternalInput").ap()
    WkT = nc.dram_tensor("WkT", [E, TWO_D], F16, kind="ExternalInput").ap()
    WvT = nc.dram_tensor("WvT", [E, D], F16, kind="ExternalInput").ap()
    bqc = nc.dram_tensor("bqc", [P, N_F], F32, kind="ExternalInput").ap()
    bkc = nc.dram_tensor("bkc", [P, N_F], F32, kind="ExternalInput").ap()
    bvb = nc.dram_tensor("bvb", [P, D], F32, kind="ExternalInput").ap()
    plam = nc.dram_tensor("plam", [P, 1], F32, kind="ExternalInput").ap()
    eye = nc.dram_tensor("eye", [P, P], F16, kind="ExternalInput").ap()
    out = nc.dram_tensor("out", [QR, D], F32, kind="ExternalOutput").ap()

    const = ctx.enter_context(tc.tile_pool(name="const", bufs=1))
    qkv = ctx.enter_context(tc.tile_pool(name="qkv", bufs=1))
    rp = ctx.enter_context(tc.tile_pool(name="rp", bufs=4))
    finp = ctx.enter_context(tc.tile_pool(name="finp", bufs=2))
    ps_work = ctx.enter_context(tc.tile_pool(name="ps_work", bufs=3, space="PSUM"))
    ps_out = ctx.enter_context(tc.tile_pool(name="ps_out", bufs=4, space="PSUM"))
    ps_small = ctx.enter_context(tc.tile_pool(name="ps_small", bufs=1, space="PSUM"))

    for rep in range(reps):
        # ---- constants ----
        bqc_sb = const.tile([P, N_F], F32, tag="bqc")
        bkc_sb = const.tile([P, N_F], F32, tag="bkc")
        bvb_sb = const.tile([P, D], F32, tag="bvb")
        plam_sb = const.tile([P, 1], F32, tag="plam")
        eye_sb = const.tile([P, P], F16, tag="eye")
        nc.scalar.dma_start(bqc_sb[:], bqc[:])
        nc.scalar.dma_start(bkc_sb[:], bkc[:])
        nc.scalar.dma_start(bvb_sb[:], bvb[:])
        nc.scalar.dma_start(plam_sb[:], plam[:])
        nc.scalar.dma_start(eye_sb[:], eye[:])
        ones_col = const.tile([P, 1], F16, tag="ones_col")
        nc.vector.memset(ones_col[:], 1.0)

        # ---- resident Q/K/V ----
        qt = qkv.tile([P, N_F, QR], F16, tag="qt")   # [f, q]
        kt = qkv.tile([P, N_F, S], F16, tag="kt")    # [f, k]
        vt = qkv.tile([P, KC, D], F16, tag="vt")     # [k, d]

        # ---- projections ----
        with tc.tile_pool(name=f"wp{rep}", bufs=1) as wp, tc.tile_pool(
            name=f"xtp{rep}", bufs=2
        ) as xtp:
            wkt = wp.tile([P, N_E, TWO_D], F16, tag="wkt")  # [e, f]
            wvt = wp.tile([P, N_E, D], F16, tag="wvt")
            wqt = wp.tile([P, N_E, TWO_D], F16, tag="wqt")
            for eo in range(N_E):
                nc.sync.dma_start(wkt[:, eo, :], WkT[eo * P : (eo + 1) * P, :])
            for eo in range(N_E):
                nc.scalar.dma_start(wvt[:, eo, :], WvT[eo * P : (eo + 1) * P, :])
            for eo in range(N_E):
                nc.scalar.dma_start(wqt[:, eo, :], WqT[eo * P : (eo + 1) * P, :])

            for sb in range(S // QB):
                xt_blk = xtp.tile([P, N_E, QB], F16, tag="xt")
                for eo in range(N_E):
                    nc.sync.dma_start(
                        xt_blk[:, eo, :],
                        XT[eo * P : (eo + 1) * P, sb * QB : (sb + 1) * QB],
                    )
                for fo in range(N_F):
                    ps = ps_work.tile([P, QB], F32, tag="work")
                    for eo in range(N_E):
                        nc.tensor.matmul(
                            ps[:],
                            wkt[:, eo, fo * P : (fo + 1) * P],
                            xt_blk[:, eo, :],
                            start=eo == 0,
                            stop=eo == N_E - 1,
                        )
                    nc.scalar.activation(
                        kt[:, fo, sb * QB : (sb + 1) * QB],
                        ps[:],
                        AF.Identity,
                        bias=bkc_sb[:, fo : fo + 1],
                    )
                for ss in range(QB // P):
                    ps = ps_work.tile([P, QB], F32, tag="work")
                    for eo in range(N_E):
                        nc.tensor.matmul(
                            ps[:],
                            xt_blk[:, eo, ss * P : (ss + 1) * P],
                            wvt[:, eo, :],
                            start=eo == 0,
                            stop=eo == N_E - 1,
                        )
                    nc.vector.tensor_tensor(
                        vt[:, sb * (QB // P) + ss, :], ps[:], bvb_sb[:], ALU.add
                    )
                if sb < QR // QB:  # query rows are the prefix (host-rotated)
                    for fo in range(N_F):
                        ps = ps_work.tile([P, QB], F32, tag="work")
                        for eo in range(N_E):
                            nc.tensor.matmul(
                                ps[:],
                                wqt[:, eo, fo * P : (fo + 1) * P],
                                xt_blk[:, eo, :],
                                start=eo == 0,
                                stop=eo == N_E - 1,
                            )
                        nc.scalar.activation(
                            qt[:, fo, sb * QB : (sb + 1) * QB],
                            ps[:],
                            AF.Identity,
                            bias=bqc_sb[:, fo : fo + 1],
                        )

        # ---- attention ----
        # Per q-block: E1/E2 resident, DVE row-sum acc; fold sums via ones
        # matmuls sharing one PSUM bank group; D = E1 - LAM*E2 (LAM[p,q] =
        # lam*s1[q]/s2[q] broadcast over partitions); single P@V pass; final
        # per-partition 1/s1 scale.
        with tc.tile_pool(name=f"ep{rep}", bufs=1) as ep:
            for qb in range(QR // QB):
                e1 = ep.tile([P, KC, QB], F16, tag="e1")
                e2 = ep.tile([P, KC, QB], F16, tag="e2")
                acc1 = ep.tile([P, QB], F16, tag="acc1")
                acc2 = ep.tile([P, QB], F16, tag="acc2")
                outp = [
                    ps_out.tile([P, D], F32, tag="out", name=f"out{qs}")
                    for qs in range(NQS)
                ]
                for m, (ebuf, accm) in enumerate(((e1, acc1), (e2, acc2))):
                    for kc in range(KC):
                        a_ps = ps_work.tile([P, QB], F32, tag="work")
                        for dd in range(4):
                            fo = m * 4 + dd
                            nc.tensor.matmul(
                                a_ps[:],
                                kt[:, fo, kc * P : (kc + 1) * P],
                                qt[:, fo, qb * QB : (qb + 1) * QB],
                                start=dd == 0,
                                stop=dd == 3,
                            )
                        nc.scalar.activation(
                            ebuf[:, kc, :], a_ps[:], AF.Exp, scale=S_SCALE
                        )
                        if kc == 0:
                            nc.vector.tensor_copy(accm[:], ebuf[:, kc, :])
                        else:
                            nc.vector.tensor_tensor(
                                accm[:], accm[:], ebuf[:, kc, :], ALU.add
                            )

                # row sums: one PSUM bank, cols 0..3 = s1 per qs, 4..7 = s2
                sums = ps_small.tile([P, QB], F32, tag="small")
                for m, accm in ((0, acc1), (1, acc2)):
                    for qs in range(NQS):
                        nc.tensor.matmul(
                            sums[:, m * NQS + qs : m * NQS + qs + 1],
                            accm[:, qs * P : (qs + 1) * P],
                            ones_col[:],
                            start=(m == 0 and qs == 0),
                            stop=(m == 1 and qs == NQS - 1),
                        )
                r1 = rp.tile([P, NQS], F32, tag="r1")
                gcol = rp.tile([P, NQS], F16, tag="gcol")
                for qs in range(NQS):
                    nc.vector.reciprocal(r1[:, qs : qs + 1], sums[:, qs : qs + 1])
                    r2 = rp.tile([P, 1], F32, tag="r2")
                    nc.vector.reciprocal(r2[:], sums[:, NQS + qs : NQS + qs + 1])
                    t = rp.tile([P, 1], F32, tag="t")
                    nc.vector.tensor_tensor(
                        t[:], sums[:, qs : qs + 1], r2[:], ALU.mult
                    )
                    nc.vector.tensor_scalar(
                        gcol[:, qs : qs + 1], t[:], plam_sb[:, 0:1], None, ALU.mult
                    )
                # transpose g columns into a [1, QB] row (one PSUM group),
                # then broadcast to all partitions
                grow_ps = ps_small.tile([P, QB], F32, tag="small")
                for qs in range(NQS):
                    nc.tensor.matmul(
                        grow_ps[0:1, qs * P : (qs + 1) * P],
                        gcol[:, qs : qs + 1],
                        eye_sb[:],
                        start=qs == 0,
                        stop=qs == NQS - 1,
                    )
                grow = rp.tile([1, QB], F16, tag="grow")
                nc.vector.tensor_copy(grow[:], grow_ps[0:1, :])
                lamb = ep.tile([P, QB], F16, tag="lamb")
                nc.gpsimd.partition_broadcast(lamb[:], grow[:])

                # combine + single P@V pass
                for kc in range(KC):
                    nc.vector.tensor_tensor(
                        e2[:, kc, :], e2[:, kc, :], lamb[:], ALU.mult
                    )
                    nc.vector.tensor_tensor(
                        e1[:, kc, :], e1[:, kc, :], e2[:, kc, :], ALU.subtract
                    )
                    for qs in range(NQS):
                        nc.tensor.matmul(
                            outp[qs][:],
                            e1[:, kc, qs * P : (qs + 1) * P],
                            vt[:, kc, :],
                            start=kc == 0,
                            stop=kc == KC - 1,
                        )
                for qs in range(NQS):
                    fin = finp.tile([P, D], F32, tag="fin")
                    nc.vector.tensor_scalar(
                        fin[:], outp[qs][:], r1[:, qs : qs + 1], None, ALU.mult
                    )
                    row0 = qb * QB + qs * P
                    nc.sync.dma_start(out[row0 : row0 + P, :], fin[:])


_NC_CACHE = {}


def _get_nc(reps=1):
    if reps not in _NC_CACHE:
        nc = bacc.Bacc("TRN2", target_bir_lowering=False, debug=False, num_devices=8)
        with tile.TileContext(nc) as tc:
            with __import__("contextlib").ExitStack() as ctx:
                _emit(nc, tc, ctx, reps=reps)
        nc.compile()
        _NC_CACHE[reps] = nc
    return _NC_CACHE[reps]


def kernel(X, Wq, bq, Wk, bk, Wv, bv, lam, **_unused):
    global LAST_RESULTS
    X = np.asarray(X, dtype=np.float32)
    Wq = np.asarray(Wq, dtype=np.float32)
    Wk = np.asarray(Wk, dtype=np.float32)
    Wv = np.asarray(Wv, dtype=np.float32)
    bq_ = np.asarray(bq, dtype=np.float32).reshape(N_F, P)
    bk_ = np.asarray(bk, dtype=np.float32).reshape(N_F, P)
    bv_ = np.asarray(bv, dtype=np.float32).reshape(1, D)
    lam_ = np.asarray(lam, dtype=np.float32).reshape(())

    WqT16 = np.ascontiguousarray(Wq.astype(np.float16).T)
    WkT16 = np.ascontiguousarray(Wk.astype(np.float16).T)
    WvT16 = np.ascontiguousarray(Wv.astype(np.float16).T)
    bqc_ = np.ascontiguousarray(bq_.T)
    bkc_ = np.ascontiguousarray(bk_.T)
    bvb_ = np.ascontiguousarray(np.broadcast_to(bv_, (P, D)))
    plam_ = np.full((P, 1), np.exp(lam_) + LAMBDA_INIT, dtype=np.float32)
    eye16 = np.eye(P, dtype=np.float16)

    xt16 = [np.ascontiguousarray(X[b].astype(np.float16).T) for b in range(B)]

    nc = _get_nc()
    in_maps = []
    for c in range(8):
        b, h = c // 2, c % 2
        if h == 0:
            xt_c = xt16[b]
        else:
            xt_c = np.ascontiguousarray(
                np.concatenate([xt16[b][:, QR:], xt16[b][:, :QR]], axis=1)
            )
        in_maps.append(
            {
                "XT": xt_c,
                "WqT": WqT16,
                "WkT": WkT16,
                "WvT": WvT16,
                "bqc": bqc_,
                "bkc": bkc_,
                "bvb": bvb_,
                "plam": plam_,
                "eye": eye16,
            }
        )
    trace = bool(int(os.environ.get("DIFFATTN_TRACE", "0")))
    res = run_bass_kernel_spmd(nc, in_maps, core_ids=list(range(8)), trace=trace)
    LAST_RESULTS = res
    full = np.empty((B, S, D), dtype=np.float32)
    for c in range(8):
        b, h = c // 2, c % 2
        full[b, h * QR : (h + 1) * QR] = res.results[c]["out"]
    return full
